# revision 13
# baseline (speedup 1.0000x reference)
"""Trainium2 Bass kernel v3 for pointer-generator additive attention.

v2 (transposed pass-1, PE score-reduce) reached HW correctness but ran at
~188us: the PE never held its 2.4 GHz p-state because att(k,c) WAR-stalled
on tanh(k,c-2) every slot (2 PSUM att buffers) and score(k,c-2) waits were
satisfied just-in-time.  A back-to-back microbench shows the PE sustains
218 ns per 512-col matmul (full clock) and fp8 DoubleRow streams TWO
k-tiles in that same 218 ns, with LDWEIGHTS fully hidden.

v3 keeps v2's math but rebuilds the pipeline for PE continuity:
  - 3 att PSUM buffers [128,1024] (6 banks) -- att(k,c) now WARs on
    tanh(k,c-3), finished ~2 slots earlier.  The PSUM transpose scratch
    that blocked the 3rd buffer is gone:
      * attn one-hot columns come from a DMA xbar transpose
        ([16,128] bf16 -> [128,16]; aT[p,c] = attn[c*128+p]), fed by a
        bf16 copy of attn produced inside softmax.
      * dec transposes go through a borrowed att_ps[2] column region
        at startup, drained by ONE [128,64] DVE copy into decT.
  - score lag deepened to 4 slots; th ring = 6 bufs.
  - c_t lag 3 for batches 0-5; {6,7} get singleton softmaxes (batch 6 at
    packed row 96, batch 7 in standalone tiles) so ct(6) runs step 8 and
    ct(7) step 9 -- tail ~12us instead of ~34us.
  - ACT table preloaded via a dummy tanh at t=0.
  - shared PSUM [98,1024]: c_t rows 0-7, score rows 32/64 by parity.
"""

import sys

if "/opt/trn_rl_repo" not in sys.path:
    sys.path.insert(0, "/opt/trn_rl_repo")

import ml_dtypes
import numpy as np

import concourse.bass as bass
import concourse.mybir as mybir
import concourse.tile as tile
from concourse import bacc
from concourse.bass_utils import run_bass_kernel_spmd
from concourse.masks import make_identity

F32 = mybir.dt.float32
BF16 = mybir.dt.bfloat16
FP8 = mybir.dt.float8e4
AF = mybir.ActivationFunctionType
ALU = mybir.AluOpType
DR = mybir.MatmulPerfMode.DoubleRow

CUR = [""]   # emission label, for schedule debugging

N_CORES = 8
B = 64
NB = B // N_CORES  # 8 local batches per core
T = 1024
N = 1024
NCH = 8            # n-chunks of 128 per batch


def build_bass(nb: int = NB) -> bass.Bass:
    nc = bacc.Bacc()

    ef_d = nc.declare_dram_parameter("ef_blk", [nb, 9, 128, T], FP8, isOutput=False)
    lhs_d = nc.declare_dram_parameter("lhs_blk", [128, 9, 128], FP8, isOutput=False)
    eo_d = nc.declare_dram_parameter("eo_blk", [nb, 8, 128, N], BF16, isOutput=False)
    mk_d = nc.declare_dram_parameter("enc_padding_mask", [nb, T], F32, isOutput=False)
    cv_d = nc.declare_dram_parameter("coverage", [nb, T], F32, isOutput=False)
    wdt_d = nc.declare_dram_parameter("W_d_T", [4, 128, 2048], BF16, isOutput=False)
    st_d = nc.declare_dram_parameter("s_t_hat_T", [N, nb], BF16, isOutput=False)
    bd_d = nc.declare_dram_parameter("b_d", [N], BF16, isOutput=False)
    vt_d = nc.declare_dram_parameter("v_T", [128, 8], BF16, isOutput=False)
    id_d = nc.declare_dram_parameter("ident8", [8, 8], F32, isOutput=False)
    ct_o = nc.declare_dram_parameter("c_t", [nb, N], F32, isOutput=True)
    at_o = nc.declare_dram_parameter("attn", [nb, T], F32, isOutput=True)
    cn_o = nc.declare_dram_parameter("coverage_next", [nb, T], F32, isOutput=True)

    with tile.TileContext(nc) as tc:
        with (
            tc.tile_pool(name="consts", bufs=1) as consts,
            tc.tile_pool(name="efp", bufs=1) as efp,
            tc.tile_pool(name="eop", bufs=1) as eop,
            tc.tile_pool(name="thp", bufs=1) as thp,
            tc.tile_pool(name="psA", bufs=1, space="PSUM") as psA,
            tc.tile_pool(name="psS", bufs=1, space="PSUM") as psS,
        ):
            # ---------------- SBUF tiles ----------------
            ef_bufs = [efp.tile([128, 9, T], FP8, name=f"ef{i}") for i in range(3)]
            eo_bufs = [eop.tile([128, 8, N], BF16, name=f"eo{i}") for i in range(5)]
            th_bufs = [thp.tile([128, N], BF16, name=f"th{i}") for i in range(6)]

            lhs_t = consts.tile([128, 9, 128], FP8)
            sT_all = consts.tile([128, NCH, NB], BF16)
            wpairs = [consts.tile([128, 2, N], BF16, name=f"wp{i}") for i in range(4)]
            bd_b = consts.tile([1, N], BF16)
            vt_sb = consts.tile([128, 8], BF16)
            ones8 = consts.tile([1, NB], BF16)
            ident8 = consts.tile([8, 8], F32)
            dec_rows = consts.tile([NB, N], F32)
            decT = consts.tile([128, 8, NB], BF16)
            dummy = consts.tile([1, 1], BF16)
            # singleton softmax: batch b lives at row 32*(b%4) of set b//4
            # (32-aligned partition bases for engine ops)
            mask_s = [consts.tile([98, T], F32, name=f"mk{i}") for i in range(2)]
            covf_s = [consts.tile([98, T], F32, name=f"cf{i}") for i in range(2)]
            attn_s = [consts.tile([98, T], F32, name=f"at{i}") for i in range(2)]
            attnb_s = [consts.tile([98, T], BF16, name=f"ab{i}") for i in range(2)]
            covn_s = [consts.tile([98, T], F32, name=f"cn{i}") for i in range(2)]
            ssum_s = [consts.tile([98, 1], F32, name=f"ss{i}") for i in range(2)]
            rs_s = [consts.tile([98, 1], F32, name=f"rs{i}") for i in range(2)]
            a16 = [consts.tile([16, 128], BF16, name=f"a16_{i}") for i in range(2)]
            aT16 = [consts.tile([128, 16], BF16, name=f"aT16_{i}") for i in range(2)]
            acw = [consts.tile([128, 8, NB], BF16, name=f"acw{b}") for b in range(NB)]
            ct_sb = consts.tile([NB, N], F32)

            # ---------------- PSUM tiles: 6 + 2 = 8 banks ----------------
            att_ps = [psA.tile([128, N], F32, name=f"att{i}") for i in range(3)]
            shared = psS.tile([98, N], F32)   # ct rows 0-7, score @32 / @64

            # ---------------- startup DMAs ----------------
            nc.sync.dma_start(out=sT_all, in_=st_d.rearrange("(c p) b -> p c b", p=128))
            for c2 in range(4):
                for kk in range(2):
                    q = nc.sync if kk == 0 else nc.gpsimd
                    q.dma_start(
                        out=wpairs[c2][:, kk, :],
                        in_=wdt_d[c2, :, :].rearrange(
                            "p (k n) -> p k n", k=2)[:, kk, :],
                    )
            nc.sync.dma_start(out=bd_b, in_=bd_d[None, :])
            nc.sync.dma_start(out=vt_sb, in_=vt_d[:, :])
            nc.gpsimd.dma_start(out=lhs_t, in_=lhs_d[:, :, :])

            nc.sync.dma_start(out=ident8, in_=id_d[:, :])
            for b in range(4):
                nc.gpsimd.dma_start(
                    out=mask_s[0][32 * b:32 * b + 1, :], in_=mk_d[b:b + 1, :]
                )
                nc.gpsimd.dma_start(
                    out=covf_s[0][32 * b:32 * b + 1, :], in_=cv_d[b:b + 1, :]
                )
            nc.vector.memset(ones8, 1.0)
            # ACT table preload off the critical path
            nc.scalar.activation(dummy, ones8[0:1, 0:1], AF.Tanh)
            for b in range(NB):
                nc.vector.memset(acw[b], 0.0)
            for i in range(2):
                nc.vector.memset(a16[i], 0.0)

            # ---------------- DMA helpers ----------------
            def load_ef(b):
                CUR[0] = f'EF{b}'
                buf = ef_bufs[b % 3]
                nc.sync.dma_start(
                    out=buf,
                    in_=ef_d[b, :, :, :].rearrange("c p t -> p c t"),
                )

            def load_eo(b):
                CUR[0] = f'EO{b}'
                buf = eo_bufs[b % 5]
                nc.sync.dma_start(
                    out=buf, in_=eo_d[b, :, :, :].rearrange("c p n -> p c n")
                )

            # ---------------- compute emitters ----------------
            def emit_att(b, c):
                CUR[0] = f'att{b}_{c}'
                buf = ef_bufs[b % 3]
                ps = att_ps[(8 * b + c) % 3]
                lhs_ap = lhs_t[:, 0:c + 2:(c + 1), :]
                for h in range(2):
                    nc.tensor.matmul(
                        ps[:, h * 512:(h + 1) * 512],
                        lhsT=lhs_ap,
                        rhs=buf[:, c:9:(8 - c), h * 512:(h + 1) * 512],
                        perf_mode=DR, start=True, stop=True,
                        skip_group_check=True,
                    )

            def emit_tanh(b, c):
                CUR[0] = f'tanh{b}_{c}'
                nc.scalar.activation(
                    th_bufs[(8 * b + c) % 6], att_ps[(8 * b + c) % 3], AF.Tanh,
                    bias=decT[:, c, b:b + 1],
                )

            def score_reg(b):
                return shared[32:33, :] if b % 2 == 0 else shared[64:65, :]

            def emit_score(b, c):
                CUR[0] = f'score{b}_{c}'
                th = th_bufs[(8 * b + c) % 6]
                reg = score_reg(b)
                for h in range(2):
                    nc.tensor.matmul(
                        reg[:, h * 512:(h + 1) * 512],
                        lhsT=vt_sb[:, c:c + 1],
                        rhs=th[:, h * 512:(h + 1) * 512],
                        start=(c == 0), stop=(c == 7),
                        skip_group_check=True,
                    )

            def emit_softmax(b):
                CUR[0] = f'softmax{b}'
                i, g = b // 4, slice(32 * (b % 4), 32 * (b % 4) + 1)
                ag, ab = attn_s[i][g, :], attnb_s[i][g, :]
                # exp straight from the score PSUM region: no drain, no gather
                nc.scalar.activation(ag, score_reg(b), AF.Exp)
                nc.vector.scalar_tensor_tensor(
                    out=ag, in0=ag, scalar=1.0, in1=mask_s[i][g, :],
                    op0=ALU.mult, op1=ALU.mult, accum_out=ssum_s[i][g, :],
                )
                nc.vector.reciprocal(rs_s[i][g, :], ssum_s[i][g, :])
                nc.vector.tensor_scalar_mul(ag, ag, rs_s[i][g, :])
                nc.vector.tensor_copy(ab, ag)
                nc.vector.tensor_add(
                    covn_s[i][g, :], covf_s[i][g, :], ag
                )
                nc.gpsimd.dma_start(out=at_o[b:b + 1, :], in_=ag)
                nc.gpsimd.dma_start(out=cn_o[b:b + 1, :], in_=covn_s[i][g, :])

            def emit_acw(b):
                CUR[0] = f'acw{b}'
                i, r = b // 4, 32 * (b % 4)
                t16 = a16[b % 2]
                # regroup + xbar both on sync, issued ahead of the step's bulk
                nc.sync.dma_start(
                    out=t16[0:8, :],
                    in_=attnb_s[i][r:r + 1, :].rearrange("p (c t) -> p c t", c=8),
                )
                aT = aT16[b % 2]
                nc.sync.dma_start_transpose(out=aT, in_=t16)
                nc.vector.tensor_copy(acw[b][:, :, b], aT[:, 0:8])

            def emit_ct(b, c):
                CUR[0] = f'ct{b}_{c}'
                buf = eo_bufs[b % 5]
                for h in range(2):
                    nc.tensor.matmul(
                        shared[0:8, h * 512:(h + 1) * 512],
                        lhsT=acw[b][:, c, :],
                        rhs=buf[:, c, h * 512:(h + 1) * 512],
                        start=(b == 0 and c == 0),
                        stop=(b == NB - 1 and c == 7),
                        skip_group_check=True,
                    )

            def emit_dec():
                CUR[0] = 'dec'
                dv = att_ps[1]   # matvec accumulator (rows 0-7)
                sc = att_ps[2]   # transpose scratch (cols 0-63)
                dT = decT.rearrange("p c b -> p (c b)")
                # h-split: n-halves pipelined so tanh(0,0) starts ~4us sooner
                for h in range(2):
                    for kj in range(NCH):
                        nc.tensor.matmul(
                            dv[0:8, h * 512:(h + 1) * 512],
                            lhsT=sT_all[:, kj, :],
                            rhs=wpairs[kj // 2][:, kj % 2, h * 512:(h + 1) * 512],
                            start=(kj == 0), stop=False, skip_group_check=True,
                        )
                    nc.tensor.matmul(
                        dv[0:8, h * 512:(h + 1) * 512],
                        lhsT=ones8, rhs=bd_b[0:1, h * 512:(h + 1) * 512],
                        start=False, stop=True, skip_group_check=True,
                    )
                    nc.vector.tensor_copy(
                        dec_rows[:, h * 512:(h + 1) * 512],
                        dv[0:8, h * 512:(h + 1) * 512],
                    )
                    for c in range(4 * h, 4 * h + 4):
                        nc.tensor.matmul(
                            sc[:, c * 8:(c + 1) * 8],
                            lhsT=dec_rows[:, c * 128:(c + 1) * 128],
                            rhs=ident8[0:8, 0:8],
                            is_transpose=True, start=True, stop=True,
                            skip_group_check=True,
                        )
                    nc.vector.tensor_copy(
                        dT[:, 32 * h:32 * h + 32],
                        sc[:, 32 * h:32 * h + 32],
                    )

            # ---------------- schedule ----------------
            load_ef(0)
            load_ef(1)
            emit_att(0, 0)
            emit_dec()

            for k in range(10):
                for c in range(8):
                    if k <= 7 and not (k == 0 and c == 0) \
                            and not (k >= 1 and c <= 1):
                        emit_att(k, c)
                    if k <= 7:
                        emit_tanh(k, c)
                    # score lag 4: slots 0-3 finish batch k-1, 4-7 run batch
                    # k; batch 7 runs at lag 2 so the tail starts sooner
                    if c < 4:
                        sb_, sc_ = k - 1, 4 + c
                    else:
                        sb_, sc_ = k, c - 4
                    if 0 <= sb_ <= 6 and (sb_ == k - 1 or k <= 7):
                        emit_score(sb_, sc_)
                    if k == 7 and c >= 2:
                        emit_score(7, c - 2)
                    if c == 6 and k <= 6:
                        emit_att(k + 1, 0)
                    if c == 7 and k <= 6:
                        emit_att(k + 1, 1)
                    if c == 7 and 1 <= k <= 7:
                        emit_softmax(k - 1)
                        emit_acw(k - 1)
                    # c_t: lag 4 (acw chain gets ~1.5 steps of slack)
                    if 4 <= k <= 7:
                        emit_ct(k - 4, c)
                # tail: batch-7 softmax chain first, then remaining c_t
                if k == 8:
                    emit_score(7, 6)
                    emit_score(7, 7)
                    emit_softmax(7)
                    emit_acw(7)
                    for b_ in (4, 5, 6):
                        for c_ in range(8):
                            emit_ct(b_, c_)
                if k == 9:
                    for c_ in range(8):
                        emit_ct(7, c_)
                # bulk loads at end of step: latency-critical smalls (score
                # gather, acw xbar) issued mid-step go ahead of them in the
                # sync queue; prefetch depth still covers arrival
                if k + 2 <= NB - 1:
                    load_ef(k + 2)
                # EO shifted one step early: WAR on buf (k+1)%4 vs ct(k-3)
                # is emitted just above, and the tail then has no bulk DMA
                # blocking the acw smalls
                if k == 0:
                    load_eo(0)
                    load_eo(1)
                elif k + 1 <= NB - 1:
                    load_eo(k + 1)
                if k == 0:
                    for b in range(4, 8):
                        nc.gpsimd.dma_start(
                            out=mask_s[1][32 * (b % 4):32 * (b % 4) + 1, :],
                            in_=mk_d[b:b + 1, :],
                        )
                        nc.gpsimd.dma_start(
                            out=covf_s[1][32 * (b % 4):32 * (b % 4) + 1, :],
                            in_=cv_d[b:b + 1, :],
                        )

            nc.vector.tensor_copy(ct_sb, shared[0:8, :])
            nc.sync.dma_start(out=ct_o[:, :], in_=ct_sb)

    nc.finalize()
    return nc


_CACHE: dict = {}


def _get_nc() -> bass.Bass:
    if "nc" not in _CACHE:
        _CACHE["nc"] = build_bass(NB)
    return _CACHE["nc"]


def make_in_maps(inputs: dict) -> list:
    f = lambda x: np.ascontiguousarray(np.asarray(x), dtype=np.float32)
    s = f(inputs["s_t_hat"])
    eo = f(inputs["encoder_outputs"])
    ef = f(inputs["encoder_feature"]).reshape(B, T, N)
    mk = f(inputs["enc_padding_mask"])
    cv = f(inputs["coverage"])
    wdt = np.ascontiguousarray(f(inputs["W_d"]).T).astype(ml_dtypes.bfloat16)
    wdt_pairs = np.ascontiguousarray(
        wdt.reshape(8, 128, N).transpose(1, 0, 2).reshape(128, 4, 2048)
        .transpose(1, 0, 2)
    )
    bd = f(inputs["b_d"]).astype(ml_dtypes.bfloat16)
    wc = f(inputs["W_c"])
    vv = f(inputs["v"])
    vt = np.ascontiguousarray(vv.reshape(8, 128).T).astype(ml_dtypes.bfloat16)

    lhs = np.zeros((128, 9, 128), np.float32)
    lhs[:, 0, :] = np.eye(128, dtype=np.float32)
    for c in range(8):
        lhs[0, c + 1, :] = wc[c * 128:(c + 1) * 128]
    lhs_blk = lhs.astype(ml_dtypes.float8_e4m3)

    in_maps = []
    for i in range(N_CORES):
        sl = slice(i * NB, (i + 1) * NB)
        ef_blk = np.zeros((NB, 9, 128, T), ml_dtypes.float8_e4m3)
        ef_blk[:, 0:8] = ef[sl].transpose(0, 2, 1).reshape(
            NB, NCH, 128, T).astype(ml_dtypes.float8_e4m3)
        ef_blk[:, 8, 0, :] = cv[sl].astype(ml_dtypes.float8_e4m3)
        eo_blk = np.ascontiguousarray(
            eo[sl].reshape(NB, 8, 128, N)
        ).astype(ml_dtypes.bfloat16)
        in_maps.append({
            "ef_blk": ef_blk,
            "lhs_blk": lhs_blk,
            "eo_blk": eo_blk,
            "enc_padding_mask": mk[sl],
            "coverage": cv[sl],
            "W_d_T": wdt_pairs,
            "s_t_hat_T": np.ascontiguousarray(s[sl].T).astype(ml_dtypes.bfloat16),
            "b_d": bd,
            "v_T": vt,
            "ident8": np.eye(8, dtype=np.float32),
        })
    return in_maps


def gather_outputs(results: list):
    c_t = np.concatenate([results[i]["c_t"] for i in range(N_CORES)], axis=0)
    attn = np.concatenate([results[i]["attn"] for i in range(N_CORES)], axis=0)
    covn = np.concatenate(
        [results[i]["coverage_next"] for i in range(N_CORES)], axis=0
    )
    return c_t, attn, covn


def kernel(**inputs):
    nc = _get_nc()
    in_maps = make_in_maps(inputs)
    res = run_bass_kernel_spmd(nc, in_maps, core_ids=list(range(N_CORES)))
    return gather_outputs(res.results)


# revision 14
# speedup vs baseline: 1.0167x; 1.0167x over previous
"""Trainium2 Bass kernel v3 for pointer-generator additive attention.

v2 (transposed pass-1, PE score-reduce) reached HW correctness but ran at
~188us: the PE never held its 2.4 GHz p-state because att(k,c) WAR-stalled
on tanh(k,c-2) every slot (2 PSUM att buffers) and score(k,c-2) waits were
satisfied just-in-time.  A back-to-back microbench shows the PE sustains
218 ns per 512-col matmul (full clock) and fp8 DoubleRow streams TWO
k-tiles in that same 218 ns, with LDWEIGHTS fully hidden.

v3 keeps v2's math but rebuilds the pipeline for PE continuity:
  - 3 att PSUM buffers [128,1024] (6 banks) -- att(k,c) now WARs on
    tanh(k,c-3), finished ~2 slots earlier.  The PSUM transpose scratch
    that blocked the 3rd buffer is gone:
      * attn one-hot columns come from a DMA xbar transpose
        ([16,128] bf16 -> [128,16]; aT[p,c] = attn[c*128+p]), fed by a
        bf16 copy of attn produced inside softmax.
      * dec transposes go through a borrowed att_ps[2] column region
        at startup, drained by ONE [128,64] DVE copy into decT.
  - score lag deepened to 4 slots; th ring = 6 bufs.
  - c_t lag 3 for batches 0-5; {6,7} get singleton softmaxes (batch 6 at
    packed row 96, batch 7 in standalone tiles) so ct(6) runs step 8 and
    ct(7) step 9 -- tail ~12us instead of ~34us.
  - ACT table preloaded via a dummy tanh at t=0.
  - shared PSUM [98,1024]: c_t rows 0-7, score rows 32/64 by parity.
"""

import sys

if "/opt/trn_rl_repo" not in sys.path:
    sys.path.insert(0, "/opt/trn_rl_repo")

import ml_dtypes
import numpy as np

import concourse.bass as bass
import concourse.mybir as mybir
import concourse.tile as tile
from concourse import bacc
from concourse.bass_utils import run_bass_kernel_spmd
from concourse.masks import make_identity

F32 = mybir.dt.float32
BF16 = mybir.dt.bfloat16
FP8 = mybir.dt.float8e4
AF = mybir.ActivationFunctionType
ALU = mybir.AluOpType
DR = mybir.MatmulPerfMode.DoubleRow

CUR = [""]   # emission label, for schedule debugging

N_CORES = 8
B = 64
NB = B // N_CORES  # 8 local batches per core
T = 1024
N = 1024
NCH = 8            # n-chunks of 128 per batch


def build_bass(nb: int = NB) -> bass.Bass:
    nc = bacc.Bacc()

    ef_d = nc.declare_dram_parameter("ef_blk", [nb, 9, 128, T], FP8, isOutput=False)
    lhs_d = nc.declare_dram_parameter("lhs_blk", [128, 9, 128], FP8, isOutput=False)
    eo_d = nc.declare_dram_parameter("eo_blk", [nb, 8, 128, N], BF16, isOutput=False)
    mk_d = nc.declare_dram_parameter("enc_padding_mask", [nb, T], F32, isOutput=False)
    cv_d = nc.declare_dram_parameter("coverage", [nb, T], F32, isOutput=False)
    wdt_d = nc.declare_dram_parameter("W_d_T", [4, 128, 2048], BF16, isOutput=False)
    st_d = nc.declare_dram_parameter("s_t_hat_T", [N, nb], BF16, isOutput=False)
    bd_d = nc.declare_dram_parameter("b_d", [N], BF16, isOutput=False)
    vt_d = nc.declare_dram_parameter("v_T", [128, 8], BF16, isOutput=False)
    id_d = nc.declare_dram_parameter("ident8", [8, 8], F32, isOutput=False)
    ct_o = nc.declare_dram_parameter("c_t", [nb, N], F32, isOutput=True)
    at_o = nc.declare_dram_parameter("attn", [nb, T], F32, isOutput=True)
    cn_o = nc.declare_dram_parameter("coverage_next", [nb, T], F32, isOutput=True)

    with tile.TileContext(nc) as tc:
        with (
            tc.tile_pool(name="consts", bufs=1) as consts,
            tc.tile_pool(name="efp", bufs=1) as efp,
            tc.tile_pool(name="eop", bufs=1) as eop,
            tc.tile_pool(name="thp", bufs=1) as thp,
            tc.tile_pool(name="psA", bufs=1, space="PSUM") as psA,
            tc.tile_pool(name="psS", bufs=1, space="PSUM") as psS,
        ):
            # ---------------- SBUF tiles ----------------
            ef_bufs = [efp.tile([128, 9, T], FP8, name=f"ef{i}") for i in range(3)]
            eo_bufs = [eop.tile([128, 8, N], BF16, name=f"eo{i}") for i in range(5)]
            th_bufs = [thp.tile([128, N], BF16, name=f"th{i}") for i in range(6)]

            lhs_t = consts.tile([128, 9, 128], FP8)
            sT_all = consts.tile([128, NCH, NB], BF16)
            wpairs = [consts.tile([128, 2, N], BF16, name=f"wp{i}") for i in range(4)]
            bd_b = consts.tile([1, N], BF16)
            vt_sb = consts.tile([128, 8], BF16)
            ones8 = consts.tile([1, NB], BF16)
            ident8 = consts.tile([8, 8], F32)
            dec_rows = consts.tile([NB, N], F32)
            decT = consts.tile([128, 8, NB], BF16)
            dummy = consts.tile([1, 1], BF16)
            # singleton softmax: batch b lives at row 32*(b%4) of set b//4
            # (32-aligned partition bases for engine ops)
            mask_s = [consts.tile([98, T], F32, name=f"mk{i}") for i in range(2)]
            covf_s = [consts.tile([98, T], F32, name=f"cf{i}") for i in range(2)]
            attn_s = [consts.tile([98, T], F32, name=f"at{i}") for i in range(2)]
            attnb_s = [consts.tile([98, T], BF16, name=f"ab{i}") for i in range(2)]
            covn_s = [consts.tile([98, T], F32, name=f"cn{i}") for i in range(2)]
            ssum_s = [consts.tile([98, 1], F32, name=f"ss{i}") for i in range(2)]
            rs_s = [consts.tile([98, 1], F32, name=f"rs{i}") for i in range(2)]
            a16 = [consts.tile([16, 128], BF16, name=f"a16_{i}") for i in range(2)]
            aT16 = [consts.tile([128, 16], BF16, name=f"aT16_{i}") for i in range(2)]
            acw = [consts.tile([128, 8, NB], BF16, name=f"acw{b}") for b in range(NB)]
            ct_sb = consts.tile([NB, N], F32)

            # ---------------- PSUM tiles: 6 + 2 = 8 banks ----------------
            att_ps = [psA.tile([128, N], F32, name=f"att{i}") for i in range(3)]
            shared = psS.tile([98, N], F32)   # ct rows 0-7, score @32 / @64

            # ---------------- startup DMAs ----------------
            nc.sync.dma_start(out=sT_all, in_=st_d.rearrange("(c p) b -> p c b", p=128))
            for c2 in range(4):
                for kk in range(2):
                    q = nc.sync if kk == 0 else nc.gpsimd
                    q.dma_start(
                        out=wpairs[c2][:, kk, :],
                        in_=wdt_d[c2, :, :].rearrange(
                            "p (k n) -> p k n", k=2)[:, kk, :],
                    )
            nc.sync.dma_start(out=bd_b, in_=bd_d[None, :])
            nc.sync.dma_start(out=vt_sb, in_=vt_d[:, :])
            nc.gpsimd.dma_start(out=lhs_t, in_=lhs_d[:, :, :])

            nc.sync.dma_start(out=ident8, in_=id_d[:, :])
            for b in range(4):
                nc.gpsimd.dma_start(
                    out=mask_s[0][32 * b:32 * b + 1, :], in_=mk_d[b:b + 1, :]
                )
                nc.gpsimd.dma_start(
                    out=covf_s[0][32 * b:32 * b + 1, :], in_=cv_d[b:b + 1, :]
                )
            nc.vector.memset(ones8, 1.0)
            # ACT table preload off the critical path
            nc.scalar.activation(dummy, ones8[0:1, 0:1], AF.Tanh)
            for b in range(NB):
                nc.vector.memset(acw[b], 0.0)
            for i in range(2):
                nc.vector.memset(a16[i], 0.0)

            # ---------------- DMA helpers ----------------
            def load_ef(b):
                CUR[0] = f'EF{b}'
                buf = ef_bufs[b % 3]
                nc.sync.dma_start(
                    out=buf,
                    in_=ef_d[b, :, :, :].rearrange("c p t -> p c t"),
                )

            def load_eo(b):
                CUR[0] = f'EO{b}'
                buf = eo_bufs[b % 5]
                nc.sync.dma_start(
                    out=buf, in_=eo_d[b, :, :, :].rearrange("c p n -> p c n")
                )

            # ---------------- compute emitters ----------------
            def emit_att(b, c):
                CUR[0] = f'att{b}_{c}'
                buf = ef_bufs[b % 3]
                ps = att_ps[(8 * b + c) % 3]
                lhs_ap = lhs_t[:, 0:c + 2:(c + 1), :]
                for h in range(2):
                    nc.tensor.matmul(
                        ps[:, h * 512:(h + 1) * 512],
                        lhsT=lhs_ap,
                        rhs=buf[:, c:9:(8 - c), h * 512:(h + 1) * 512],
                        perf_mode=DR, start=True, stop=True,
                        skip_group_check=True,
                    )

            def emit_tanh(b, c):
                CUR[0] = f'tanh{b}_{c}'
                nc.scalar.activation(
                    th_bufs[(8 * b + c) % 6], att_ps[(8 * b + c) % 3], AF.Tanh,
                    bias=decT[:, c, b:b + 1],
                )

            def score_reg(b):
                return shared[32:33, :] if b % 2 == 0 else shared[64:65, :]

            def emit_score(b, c):
                CUR[0] = f'score{b}_{c}'
                th = th_bufs[(8 * b + c) % 6]
                reg = score_reg(b)
                for h in range(2):
                    nc.tensor.matmul(
                        reg[:, h * 512:(h + 1) * 512],
                        lhsT=vt_sb[:, c:c + 1],
                        rhs=th[:, h * 512:(h + 1) * 512],
                        start=(c == 0), stop=(c == 7),
                        skip_group_check=True,
                    )

            def emit_softmax(b):
                CUR[0] = f'softmax{b}'
                i, g = b // 4, slice(32 * (b % 4), 32 * (b % 4) + 1)
                ag, ab = attn_s[i][g, :], attnb_s[i][g, :]
                # exp straight from the score PSUM region: no drain, no gather
                nc.scalar.activation(ag, score_reg(b), AF.Exp)
                nc.vector.scalar_tensor_tensor(
                    out=ag, in0=ag, scalar=1.0, in1=mask_s[i][g, :],
                    op0=ALU.mult, op1=ALU.mult, accum_out=ssum_s[i][g, :],
                )
                nc.vector.reciprocal(rs_s[i][g, :], ssum_s[i][g, :])
                nc.vector.tensor_scalar_mul(ag, ag, rs_s[i][g, :])
                nc.vector.tensor_copy(ab, ag)
                nc.vector.tensor_add(
                    covn_s[i][g, :], covf_s[i][g, :], ag
                )
                nc.gpsimd.dma_start(out=at_o[b:b + 1, :], in_=ag)
                nc.gpsimd.dma_start(out=cn_o[b:b + 1, :], in_=covn_s[i][g, :])

            def emit_acw(b):
                CUR[0] = f'acw{b}'
                i, r = b // 4, 32 * (b % 4)
                t16 = a16[b % 2]
                # regroup + xbar both on sync, issued ahead of the step's bulk
                nc.sync.dma_start(
                    out=t16[0:8, :],
                    in_=attnb_s[i][r:r + 1, :].rearrange("p (c t) -> p c t", c=8),
                )
                aT = aT16[b % 2]
                nc.sync.dma_start_transpose(out=aT, in_=t16)
                nc.vector.tensor_copy(acw[b][:, :, b], aT[:, 0:8])

            def emit_ct(b, c):
                CUR[0] = f'ct{b}_{c}'
                buf = eo_bufs[b % 5]
                for h in range(2):
                    nc.tensor.matmul(
                        shared[0:8, h * 512:(h + 1) * 512],
                        lhsT=acw[b][:, c, :],
                        rhs=buf[:, c, h * 512:(h + 1) * 512],
                        start=(b == 0 and c == 0),
                        stop=(b == NB - 1 and c == 7),
                        skip_group_check=True,
                    )

            def emit_dec():
                CUR[0] = 'dec'
                dv = att_ps[1]   # matvec accumulator (rows 0-7)
                sc = att_ps[2]   # transpose scratch (cols 0-63)
                dT = decT.rearrange("p c b -> p (c b)")
                # h-split: n-halves pipelined so tanh(0,0) starts ~4us sooner
                for h in range(2):
                    for kj in range(NCH):
                        nc.tensor.matmul(
                            dv[0:8, h * 512:(h + 1) * 512],
                            lhsT=sT_all[:, kj, :],
                            rhs=wpairs[kj // 2][:, kj % 2, h * 512:(h + 1) * 512],
                            start=(kj == 0), stop=False, skip_group_check=True,
                        )
                    nc.tensor.matmul(
                        dv[0:8, h * 512:(h + 1) * 512],
                        lhsT=ones8, rhs=bd_b[0:1, h * 512:(h + 1) * 512],
                        start=False, stop=True, skip_group_check=True,
                    )
                    nc.vector.tensor_copy(
                        dec_rows[:, h * 512:(h + 1) * 512],
                        dv[0:8, h * 512:(h + 1) * 512],
                    )
                    for c in range(4 * h, 4 * h + 4):
                        nc.tensor.matmul(
                            sc[:, c * 8:(c + 1) * 8],
                            lhsT=dec_rows[:, c * 128:(c + 1) * 128],
                            rhs=ident8[0:8, 0:8],
                            is_transpose=True, start=True, stop=True,
                            skip_group_check=True,
                        )
                    nc.vector.tensor_copy(
                        dT[:, 32 * h:32 * h + 32],
                        sc[:, 32 * h:32 * h + 32],
                    )

            # ---------------- schedule ----------------
            load_ef(0)
            load_ef(1)
            emit_att(0, 0)
            emit_dec()

            for k in range(10):
                for c in range(8):
                    if k <= 7 and not (k == 0 and c == 0):
                        emit_att(k, c)
                    if k <= 7:
                        emit_tanh(k, c)
                    # score lag 4: slots 0-3 finish batch k-1, 4-7 run batch
                    # k; batch 7 runs at lag 2 so the tail starts sooner
                    if c < 4:
                        sb_, sc_ = k - 1, 4 + c
                    else:
                        sb_, sc_ = k, c - 4
                    if 0 <= sb_ <= 6 and (sb_ == k - 1 or k <= 7):
                        emit_score(sb_, sc_)
                    if k == 7 and c >= 2:
                        emit_score(7, c - 2)
                    if c == 7 and 1 <= k <= 7:
                        emit_softmax(k - 1)
                        emit_acw(k - 1)
                    # c_t: lag 4 (acw chain gets ~1.5 steps of slack)
                    if 4 <= k <= 7:
                        emit_ct(k - 4, c)
                # tail: batch-7 softmax chain first, then remaining c_t
                if k == 8:
                    emit_score(7, 6)
                    emit_score(7, 7)
                    emit_softmax(7)
                    emit_acw(7)
                    for b_ in (4, 5, 6):
                        for c_ in range(8):
                            emit_ct(b_, c_)
                if k == 9:
                    for c_ in range(8):
                        emit_ct(7, c_)
                # bulk loads at end of step: latency-critical smalls (score
                # gather, acw xbar) issued mid-step go ahead of them in the
                # sync queue; prefetch depth still covers arrival
                if k + 2 <= NB - 1:
                    load_ef(k + 2)
                # EO shifted one step early: WAR on buf (k+1)%4 vs ct(k-3)
                # is emitted just above, and the tail then has no bulk DMA
                # blocking the acw smalls
                if k == 0:
                    load_eo(0)
                    load_eo(1)
                elif k + 1 <= NB - 1:
                    load_eo(k + 1)
                if k == 0:
                    for b in range(4, 8):
                        nc.gpsimd.dma_start(
                            out=mask_s[1][32 * (b % 4):32 * (b % 4) + 1, :],
                            in_=mk_d[b:b + 1, :],
                        )
                        nc.gpsimd.dma_start(
                            out=covf_s[1][32 * (b % 4):32 * (b % 4) + 1, :],
                            in_=cv_d[b:b + 1, :],
                        )

            nc.vector.tensor_copy(ct_sb, shared[0:8, :])
            nc.sync.dma_start(out=ct_o[:, :], in_=ct_sb)

    nc.finalize()
    return nc


_CACHE: dict = {}


def _get_nc() -> bass.Bass:
    if "nc" not in _CACHE:
        _CACHE["nc"] = build_bass(NB)
    return _CACHE["nc"]


def make_in_maps(inputs: dict) -> list:
    f = lambda x: np.ascontiguousarray(np.asarray(x), dtype=np.float32)
    s = f(inputs["s_t_hat"])
    eo = f(inputs["encoder_outputs"])
    ef = f(inputs["encoder_feature"]).reshape(B, T, N)
    mk = f(inputs["enc_padding_mask"])
    cv = f(inputs["coverage"])
    wdt = np.ascontiguousarray(f(inputs["W_d"]).T).astype(ml_dtypes.bfloat16)
    wdt_pairs = np.ascontiguousarray(
        wdt.reshape(8, 128, N).transpose(1, 0, 2).reshape(128, 4, 2048)
        .transpose(1, 0, 2)
    )
    bd = f(inputs["b_d"]).astype(ml_dtypes.bfloat16)
    wc = f(inputs["W_c"])
    vv = f(inputs["v"])
    vt = np.ascontiguousarray(vv.reshape(8, 128).T).astype(ml_dtypes.bfloat16)

    lhs = np.zeros((128, 9, 128), np.float32)
    lhs[:, 0, :] = np.eye(128, dtype=np.float32)
    for c in range(8):
        lhs[0, c + 1, :] = wc[c * 128:(c + 1) * 128]
    lhs_blk = lhs.astype(ml_dtypes.float8_e4m3)

    in_maps = []
    for i in range(N_CORES):
        sl = slice(i * NB, (i + 1) * NB)
        ef_blk = np.zeros((NB, 9, 128, T), ml_dtypes.float8_e4m3)
        ef_blk[:, 0:8] = ef[sl].transpose(0, 2, 1).reshape(
            NB, NCH, 128, T).astype(ml_dtypes.float8_e4m3)
        ef_blk[:, 8, 0, :] = cv[sl].astype(ml_dtypes.float8_e4m3)
        eo_blk = np.ascontiguousarray(
            eo[sl].reshape(NB, 8, 128, N)
        ).astype(ml_dtypes.bfloat16)
        in_maps.append({
            "ef_blk": ef_blk,
            "lhs_blk": lhs_blk,
            "eo_blk": eo_blk,
            "enc_padding_mask": mk[sl],
            "coverage": cv[sl],
            "W_d_T": wdt_pairs,
            "s_t_hat_T": np.ascontiguousarray(s[sl].T).astype(ml_dtypes.bfloat16),
            "b_d": bd,
            "v_T": vt,
            "ident8": np.eye(8, dtype=np.float32),
        })
    return in_maps


def gather_outputs(results: list):
    c_t = np.concatenate([results[i]["c_t"] for i in range(N_CORES)], axis=0)
    attn = np.concatenate([results[i]["attn"] for i in range(N_CORES)], axis=0)
    covn = np.concatenate(
        [results[i]["coverage_next"] for i in range(N_CORES)], axis=0
    )
    return c_t, attn, covn


def kernel(**inputs):
    nc = _get_nc()
    in_maps = make_in_maps(inputs)
    res = run_bass_kernel_spmd(nc, in_maps, core_ids=list(range(N_CORES)))
    return gather_outputs(res.results)
